# revision 14
# baseline (speedup 1.0000x reference)
"""AdaConv2d Trainium2 kernel — 8-core data-parallel (one sample per core).

Per-core pipeline (channels split into two 128-partition blocks):
  1. stream x[b] (f32) from HBM; cast into an even/odd-deinterleaved,
     reflect-padded bf16 buffer xeo [128p, 130row, 2par, 66col]; the casts
     carry accum_out so per-channel sums fall out for free; sum(x^2) runs
     on the otherwise-idle GpSimd engine. No separate normalize pass:
     instance-norm is folded into the adaptive weights (scale) and bias
     (mean correction) since the adaptive conv is linear.
  2. adaptive grouped 3x3 (+fused grouped 1x1 + instance norm) conv via
     1D Winograd F(2,3) along x: transform-domain weights U[u][dy]
     (block-diagonal 128x128) are composed on-device with bf16 matmuls
     from the Winograd-transformed spatial weights and the pointwise
     weights, then scaled in place by rstd once stats land. Per 8-row
     stripe: 4 DVE ops build V (contiguous bf16 -> 2x DVE mode), 12
     matmuls (4u x 3dy) accumulate m_u in PSUM, ACT drains to bf16, DVE
     inverse (+folded bias) writes the even/odd z buffer zeo.
  3. final dense 3x3 conv 256->256 via 1D Winograd F(2,3) along x, same
     stripe structure; inverse writes an even/odd bf16 output tile
     DMAed out per stripe (host reinterleaves + casts to f32).

Two module post-passes make the emitted program walrus-legal/fast:
  - _split_waits: walrus accepts only one embedded sync-wait per
    instruction; excess waits move to injected same-engine NOPs.
  - _dedup_ldweights: drop LDWEIGHTS that reload the already-resident
    weights.

Host side does layout-only prep (shard per-sample tensors, transpose
conv_w into lhsT layout, scatter grouped weights into block-diagonal
matrices, reinterleave the output); all arithmetic runs on device.
"""

import sys

sys.path.insert(0, "/opt/trn_rl_repo")

import ml_dtypes
import numpy as np

import concourse.bass as bass
import concourse.tile as tile
from concourse import mybir
from concourse.bass_utils import run_bass_kernel_spmd

F32 = mybir.dt.float32
BF16 = mybir.dt.bfloat16

B = 8
C = 256
H = W = 128
HW = H * W
NB = 2  # channel blocks of 128
PB = H + 2  # padded row extent (reflect pad 1)
NT = W // 2  # 64 x-tiles (F(2,3): 2 output px per tile)
NOFF = 9
EPS = 1e-5
SR = 8  # rows per stripe
NS = H // SR  # 16 stripes

_CACHE = {}
LAST_EXEC_NS = None

IDENT = mybir.ActivationFunctionType.Identity
ACOPY = mybir.ActivationFunctionType.Copy
SUB = mybir.AluOpType.subtract
ADD = mybir.AluOpType.add
MULT = mybir.AluOpType.mult


def _build():
    nc = bass.Bass(trn_type="TRN2", debug=False)

    x_d = nc.declare_dram_parameter("x", [C, HW], F32, False)
    # wcat = [wsbd (9*128) | wptbd (128) | bias (1) | convb (1)] per block
    wcat_d = nc.declare_dram_parameter("wcat", [NB, 128, NOFF * 128 + 130], BF16, False)
    cwt_d = nc.declare_dram_parameter("cwt", [NB, 128, NOFF, NB, 128], BF16, False)
    out_d = nc.declare_dram_parameter("out", [NB, 128, H, 2, NT], BF16, True)

    NCHUNK = 8  # x streamed in 16-row dma chunks
    ROWS = H // NCHUNK

    with tile.TileContext(nc) as tc:
        with (
            tc.tile_pool(name="wconst", bufs=1) as wconst,
            tc.tile_pool(name="pad", bufs=3) as padpool,
            tc.tile_pool(name="xstream", bufs=3) as xstream,
            tc.tile_pool(name="psum", bufs=8, space="PSUM") as psum,
            tc.tile_pool(name="wstage", bufs=1) as wstage,
        ):
            # ---------- (a) small weights: ONE DMA per block ------------------
            wsf = []
            wpf = []
            bias_sb = []
            convb_sb = []
            for cb in range(NB):
                wc = wstage.tile([128, NOFF * 128 + 130], BF16, name=f"wcat_{cb}")
                nc.gpsimd.dma_start(out=wc, in_=wcat_d[cb])
                wsf.append(
                    wc[:, 0 : NOFF * 128].rearrange("p (a b) -> p a b", a=NOFF)
                )
                wpf.append(wc[:, NOFF * 128 : NOFF * 128 + 128])
                bias_sb.append(wc[:, NOFF * 128 + 128 : NOFF * 128 + 129])
                convb_sb.append(wc[:, NOFF * 128 + 129 : NOFF * 128 + 130])
            # final conv weights (bf16 lhsT layout [ic, off, ocb, oc])
            cwt = []
            for icb in range(NB):
                wt = wstage.tile([128, NOFF, NB, 128], BF16, name=f"cwt_{icb}")
                nc.gpsimd.dma_start(out=wt, in_=cwt_d[icb])
                cwt.append(wt)

            eps_sb = wconst.tile([128, 1], F32, name="eps")
            nc.vector.memset(eps_sb, EPS)

            # x / z buffers: even/odd deinterleaved, padded.
            # plane 0 (even): slots 0..63 = cols 0,2,..126; slot 64 = reflect
            #   pad (dup of slot 63); slot 65 unused.
            # plane 1 (odd): slot 0 = reflect pad (dup of slot 1); slots
            #   1..64 = cols 1,3,..127; slot 65 unused.
            # row r+1 = image row r; row 0 = image row 1; row 129 = row 126.
            xeo = [
                padpool.tile([128, PB, 2, 66], BF16, tag="pad", name=f"xeo_{cb}")
                for cb in range(NB)
            ]
            zeo = [
                padpool.tile([128, PB, 2, 66], BF16, tag="pad", name=f"zeo_{cb}")
                for cb in range(NB)
            ]
            sums = [wconst.tile([128, 2 * NCHUNK], F32, name=f"sums_{cb}") for cb in range(NB)]
            sqs = [wconst.tile([128, NCHUNK], F32, name=f"sqs_{cb}") for cb in range(NB)]
            rstd = [wconst.tile([128, 1], F32, name=f"rstd_{cb}") for cb in range(NB)]
            meanb = [wconst.tile([128, 1], BF16, name=f"meanb_{cb}") for cb in range(NB)]
            biasf = [wconst.tile([128, 1], F32, name=f"biasf_{cb}") for cb in range(NB)]
            convbf = [wconst.tile([128, 1], F32, name=f"convbf_{cb}") for cb in range(NB)]
            sqscratch = wconst.tile([128, ROWS * W], BF16, name="sqscratch")

            # ---------- (b) adaptive weights. Block 0 runs the adaptive conv
            # DIRECT (9 raw composite taps; the PE has slack while block-1 x
            # still streams and DVE is busy with block-1 casts). Block 1 runs
            # 1D Winograd F(2,3): u-basis U0=W(-1), U1=(Wm+W0+Wp)/2,
            # U2=(Wm-W0+Wp)/2, U3=W(+1), built by transforming the spatial
            # weights before composing with the pointwise weights on PE.
            # PSUM drains (unscaled) are dripped between the block-0 cast
            # ops; the rstd scale is applied in place once stats land.
            us1 = [[None] * 3 for _ in range(4)]  # [u][dy] for block 1
            for dy in range(3):
                sm = wsf[1][:, dy * 3 + 0, :]
                s0 = wsf[1][:, dy * 3 + 1, :]
                sp = wsf[1][:, dy * 3 + 2, :]
                us1[0][dy] = sm
                us1[3][dy] = sp
                tmp = wstage.tile([128, 128], BF16, tag="ut", name=f"ut_{dy}")
                nc.vector.tensor_add(out=tmp, in0=sm, in1=sp)
                s0h = wstage.tile([128, 128], BF16, tag="uh", name=f"uh_{dy}")
                nc.vector.tensor_scalar_mul(out=s0h, in0=s0, scalar1=0.5)
                u1 = wconst.tile([128, 128], BF16, name=f"u1s_{dy}")
                nc.vector.scalar_tensor_tensor(
                    out=u1, in0=tmp, scalar=0.5, in1=s0h, op0=MULT, op1=ADD
                )
                u2 = wconst.tile([128, 128], BF16, name=f"u2s_{dy}")
                nc.vector.scalar_tensor_tensor(
                    out=u2, in0=tmp, scalar=0.5, in1=s0h, op0=MULT, op1=SUB
                )
                us1[1][dy] = u1
                us1[2][dy] = u2
            drip = []
            ada_w0 = [None] * NOFF  # raw taps, block 0
            for off in range(NOFF):
                ps = psum.tile([128, 128], F32, tag="ps", name=f"c0ps_{off}")
                nc.tensor.matmul(
                    ps, lhsT=wsf[0][:, off, :], rhs=wpf[0], start=True, stop=True
                )
                lt = wconst.tile([128, 128], BF16, name=f"adaw0_{off}")
                drip.append((lt, ps))
                ada_w0[off] = lt
            ada_w1 = [[None] * 3 for _ in range(4)]  # u-basis, block 1
            for u in range(4):
                for dy in range(3):
                    ps = psum.tile([128, 128], F32, tag="ps", name=f"c1ps_{u}_{dy}")
                    nc.tensor.matmul(
                        ps, lhsT=us1[u][dy], rhs=wpf[1], start=True, stop=True
                    )
                    lt = wconst.tile([128, 128], BF16, name=f"adaw1_{u}_{dy}")
                    drip.append((lt, ps))
                    ada_w1[u][dy] = lt

            # ---------- (c) x streaming --------------------------------------
            def stream_block(cb, engine):
                for ch in range(NCHUNK):
                    xc = xstream.tile(
                        [128, ROWS, W], F32, tag="xc", name=f"xc_{cb}_{ch}"
                    )
                    nc.gpsimd.dma_start(
                        out=xc,
                        in_=x_d[
                            cb * 128 : (cb + 1) * 128,
                            ch * ROWS * W : (ch + 1) * ROWS * W,
                        ],
                    )
                    r0 = 1 + ch * ROWS
                    dev = xeo[cb][:, r0 : r0 + ROWS, 0, 0:64]
                    dod = xeo[cb][:, r0 : r0 + ROWS, 1, 1:65]
                    sev = xc[:, :, 0:W:2]
                    sod = xc[:, :, 1:W:2]
                    if engine == "act":
                        nc.scalar.activation(
                            out=dev, in_=sev, func=ACOPY,
                            accum_out=sums[cb][:, 2 * ch : 2 * ch + 1],
                        )
                        nc.scalar.activation(
                            out=dod, in_=sod, func=ACOPY,
                            accum_out=sums[cb][:, 2 * ch + 1 : 2 * ch + 2],
                        )
                    else:
                        nc.vector.tensor_scalar(
                            out=dev, in0=sev, scalar1=1.0, scalar2=0.0, op0=MULT,
                            op1=ADD,
                            accum_out=sums[cb][:, 2 * ch : 2 * ch + 1],
                        )
                        nc.vector.tensor_scalar(
                            out=dod, in0=sod, scalar1=1.0, scalar2=0.0, op0=MULT,
                            op1=ADD,
                            accum_out=sums[cb][:, 2 * ch + 1 : 2 * ch + 2],
                        )
                    # sum(x^2) on DVE: out -> scratch, accum -> sqs
                    nc.vector.scalar_tensor_tensor(
                        out=sqscratch[:, 0 : ROWS * W],
                        in0=xc.rearrange("p a b -> p (a b)"),
                        scalar=1.0,
                        in1=xc.rearrange("p a b -> p (a b)"),
                        op0=MULT,
                        op1=MULT,
                        accum_out=sqs[cb][:, ch : ch + 1],
                    )
                    # drip deferred weight-compose drains between casts
                    for _ in range(3):
                        if drip:
                            dst, sp = drip.pop(0)
                            nc.scalar.copy(out=dst, in_=sp)

            def pads_eo(buf):
                # col pads (rows 1..128), then full-width row pads
                nc.scalar.copy(
                    out=buf[:, 1 : PB - 1, 0, 64:65], in_=buf[:, 1 : PB - 1, 0, 63:64]
                )
                nc.scalar.copy(
                    out=buf[:, 1 : PB - 1, 1, 0:1], in_=buf[:, 1 : PB - 1, 1, 1:2]
                )
                nc.scalar.copy(out=buf[:, 0:1, :, 0:65], in_=buf[:, 2:3, :, 0:65])
                nc.scalar.copy(
                    out=buf[:, PB - 1 : PB, :, 0:65],
                    in_=buf[:, PB - 3 : PB - 2, :, 0:65],
                )

            def stats_post(cb):
                # rstd = 1/sqrt(var+eps); var = sumsq/HW - mean^2
                s1 = wconst.tile([128, 1], F32, name=f"s1_{cb}")
                nc.vector.tensor_reduce(
                    out=s1, in_=sums[cb], axis=mybir.AxisListType.X, op=ADD
                )
                s2 = wconst.tile([128, 1], F32, name=f"s2_{cb}")
                nc.vector.tensor_reduce(
                    out=s2, in_=sqs[cb], axis=mybir.AxisListType.X, op=ADD
                )
                mean = wconst.tile([128, 1], F32, name=f"mean_{cb}")
                nc.vector.tensor_scalar_mul(out=mean, in0=s1, scalar1=1.0 / HW)
                negm = wconst.tile([128, 1], F32, name=f"negm_{cb}")
                nc.vector.tensor_scalar_mul(out=negm, in0=s1, scalar1=-1.0 / HW)
                v1 = wconst.tile([128, 1], F32, name=f"v1_{cb}")
                nc.vector.tensor_scalar_mul(out=v1, in0=s2, scalar1=1.0 / HW)
                var = wconst.tile([128, 1], F32, name=f"var_{cb}")
                nc.vector.scalar_tensor_tensor(
                    out=var, in0=mean, scalar=negm, in1=v1, op0=MULT, op1=ADD
                )
                nc.scalar.activation(
                    out=rstd[cb], in_=var,
                    func=mybir.ActivationFunctionType.Sqrt, bias=eps_sb,
                )
                nc.vector.reciprocal(out=rstd[cb], in_=rstd[cb])
                nc.vector.tensor_scalar_mul(out=meanb[cb], in0=mean, scalar1=1.0)

            def weights_post(cb):
                # scale composed weights in place by rstd (input-channel axis
                # is the partition axis of the composed tiles), then build
                # the folded bias: biasf = bias - sum_{i,off} W'[o,i,off]*mu_i.
                # Raw basis (block 0): straight sum over the 9 taps.
                # u-basis (block 1): sum_dx W = 2*U1, so corr = 2*sum_dy U1@mu.
                tiles = ada_w0 if cb == 0 else [
                    ada_w1[u][dy] for u in range(4) for dy in range(3)
                ]
                for t in tiles:
                    nc.scalar.activation(out=t, in_=t, func=ACOPY, scale=rstd[cb])
                cps = psum.tile([128, 1], F32, tag="ps", name=f"corr_{cb}")
                if cb == 0:
                    for off in range(NOFF):
                        nc.tensor.matmul(
                            cps, lhsT=ada_w0[off], rhs=meanb[0],
                            start=(off == 0), stop=(off == NOFF - 1),
                        )
                    cscale = -1.0
                else:
                    for dy in range(3):
                        nc.tensor.matmul(
                            cps, lhsT=ada_w1[1][dy], rhs=meanb[1],
                            start=(dy == 0), stop=(dy == 2),
                        )
                    cscale = -2.0
                nc.scalar.activation(
                    out=biasf[cb], in_=cps, func=IDENT,
                    scale=cscale, bias=bias_sb[cb],
                )
                nc.scalar.activation(out=convbf[cb], in_=convb_sb[cb], func=ACOPY)

            # ---------- (e) stream x ----------------------------------------
            stream_block(0, "act")
            assert not drip
            stats_post(0)
            weights_post(0)
            pads_eo(xeo[0])
            stream_block(1, "dve")
            pads_eo(xeo[1])

            # ---------- (d) final conv transform-domain weights (DVE). Emitted
            # after the streaming loops so the DVE queue services the
            # stats-critical x^2 ops first (these 72 ops would otherwise delay
            # rstd and the adaptive-conv start by ~15us).
            u12f = {}
            for dy in range(3):
                for icb in range(NB):
                    for ocb in range(NB):
                        w0 = cwt[icb][:, dy * 3 + 0, ocb, :]
                        w1 = cwt[icb][:, dy * 3 + 1, ocb, :]
                        w2 = cwt[icb][:, dy * 3 + 2, ocb, :]
                        tmp = wstage.tile([128, 128], BF16, tag="ft", name=f"ft_{dy}_{icb}_{ocb}")
                        nc.vector.tensor_add(out=tmp, in0=w0, in1=w2)
                        w1h = wstage.tile([128, 128], BF16, tag="fh", name=f"fh_{dy}_{icb}_{ocb}")
                        nc.vector.tensor_scalar_mul(out=w1h, in0=w1, scalar1=0.5)
                        u1 = wconst.tile([128, 128], BF16, name=f"fu1_{dy}_{icb}_{ocb}")
                        nc.vector.scalar_tensor_tensor(
                            out=u1, in0=tmp, scalar=0.5, in1=w1h, op0=MULT, op1=ADD
                        )
                        u2 = wconst.tile([128, 128], BF16, name=f"fu2_{dy}_{icb}_{ocb}")
                        nc.vector.scalar_tensor_tensor(
                            out=u2, in0=tmp, scalar=0.5, in1=w1h, op0=MULT, op1=SUB
                        )
                        u12f[(1, dy, icb, ocb)] = u1
                        u12f[(2, dy, icb, ocb)] = u2

            def fu_tile(u, dy, icb, ocb):
                if u == 0:
                    return cwt[icb][:, dy * 3 + 0, ocb, :]
                if u == 3:
                    return cwt[icb][:, dy * 3 + 2, ocb, :]
                return u12f[(u, dy, icb, ocb)]

            # ---------- (f) adaptive conv: 1D Winograd F(2,3), striped ------
            with (
                tc.tile_pool(name="vx", bufs=4) as vxp,
                tc.tile_pool(name="mm", bufs=3) as mmp,
                tc.tile_pool(name="ost", bufs=3) as ostp,
            ):
                def fwd_x(buf, s, nm):
                    r0 = s * SR
                    d0 = buf[:, r0 : r0 + SR + 2, 1, 0:64]
                    d1 = buf[:, r0 : r0 + SR + 2, 0, 0:64]
                    d2 = buf[:, r0 : r0 + SR + 2, 1, 1:65]
                    d3 = buf[:, r0 : r0 + SR + 2, 0, 1:65]
                    v = vxp.tile([128, 4, SR + 2, NT], BF16, tag="v", name=nm)
                    nc.vector.tensor_sub(out=v[:, 0], in0=d0, in1=d2)
                    nc.vector.tensor_add(out=v[:, 1], in0=d1, in1=d2)
                    nc.vector.tensor_sub(out=v[:, 2], in0=d2, in1=d1)
                    nc.vector.tensor_sub(out=v[:, 3], in0=d1, in1=d3)
                    return v

                def ada0_stripe(s):
                    # direct 9-tap conv for block 0: rhs taps read straight
                    # from the parity planes; even/odd output halves get
                    # separate PSUM accumulators; bias folds into the drain.
                    r0 = s * SR
                    pse = psum.tile([128, SR, NT], F32, tag="ps", name=f"a0e_{s}")
                    pso = psum.tile([128, SR, NT], F32, tag="ps", name=f"a0o_{s}")
                    src = xeo[0]
                    for off in range(NOFF):
                        dyr, dxr = off // 3 - 1, off % 3 - 1
                        ra, rb = r0 + dyr + 1, r0 + dyr + 1 + SR
                        if dxr == -1:
                            rev = src[:, ra:rb, 1, 0:64]
                            rod = src[:, ra:rb, 0, 0:64]
                        elif dxr == 0:
                            rev = src[:, ra:rb, 0, 0:64]
                            rod = src[:, ra:rb, 1, 1:65]
                        else:
                            rev = src[:, ra:rb, 1, 1:65]
                            rod = src[:, ra:rb, 0, 1:65]
                        nc.tensor.matmul(
                            pse, lhsT=ada_w0[off], rhs=rev,
                            start=(off == 0), stop=(off == NOFF - 1),
                        )
                        nc.tensor.matmul(
                            pso, lhsT=ada_w0[off], rhs=rod,
                            start=(off == 0), stop=(off == NOFF - 1),
                        )
                    z = zeo[0]
                    nc.scalar.activation(
                        out=z[:, 1 + r0 : 1 + r0 + SR, 0, 0:64], in_=pse,
                        func=IDENT, bias=biasf[0],
                    )
                    nc.scalar.activation(
                        out=z[:, 1 + r0 : 1 + r0 + SR, 1, 1:65], in_=pso,
                        func=IDENT, bias=biasf[0],
                    )

                def ada1_stripe(s, v):
                    r0 = s * SR
                    mps = [
                        psum.tile([128, SR, NT], F32, tag="ps", name=f"am_{s}_{u}")
                        for u in range(4)
                    ]
                    for u in range(4):
                        for dy in range(3):
                            nc.tensor.matmul(
                                mps[u],
                                lhsT=ada_w1[u][dy],
                                rhs=v[:, u, dy : dy + SR, :],
                                start=(dy == 0),
                                stop=(dy == 2),
                            )
                    # drain; bias folds into the m1 drain so the inverse is
                    # pure tensor_tensor (STT runs at 1x on HW, TT gets 2x):
                    #   zev = m0 + (m1+b) + m2 ; zod = (m1+b) - m2 - m3
                    msb = mmp.tile([128, 4, SR, NT], BF16, tag="m", name=f"amsb_{s}")
                    for u in range(4):
                        if u == 1:
                            nc.scalar.activation(
                                out=msb[:, 1], in_=mps[1], func=IDENT, bias=biasf[1]
                            )
                        else:
                            nc.scalar.copy(out=msb[:, u], in_=mps[u])
                    # t0/t1 on the otherwise-idle gpsimd; PE outruns DVE in
                    # this phase otherwise
                    t0 = mmp.tile([128, SR, NT], BF16, tag="t0", name=f"at0_{s}")
                    nc.gpsimd.tensor_add(out=t0, in0=msb[:, 0], in1=msb[:, 1])
                    t1 = mmp.tile([128, SR, NT], BF16, tag="t1", name=f"at1_{s}")
                    nc.gpsimd.tensor_sub(out=t1, in0=msb[:, 1], in1=msb[:, 2])
                    z = zeo[1]
                    nc.vector.tensor_add(
                        out=z[:, 1 + r0 : 1 + r0 + SR, 0, 0:64], in0=t0, in1=msb[:, 2]
                    )
                    nc.vector.tensor_sub(
                        out=z[:, 1 + r0 : 1 + r0 + SR, 1, 1:65], in0=t1, in1=msb[:, 3]
                    )

                for s in range(NS):
                    if s == 10:
                        stats_post(1)
                    if s == 11:
                        weights_post(1)
                    ada0_stripe(s)
                pads_eo(zeo[0])
                vq = fwd_x(xeo[1], 0, "av_1_0")
                for s in range(NS):
                    vn = fwd_x(xeo[1], s + 1, f"av_1_{s + 1}") if s + 1 < NS else None
                    ada1_stripe(s, vq)
                    vq = vn
                pads_eo(zeo[1])

                # ---------- (g) final conv: 1D Winograd F(2,3), striped -----
                def fin_fwd(s):
                    return [
                        fwd_x(zeo[icb], s, f"fv_{s}_{icb}") for icb in range(NB)
                    ]

                vq = fin_fwd(0)
                for s in range(NS):
                    vn = fin_fwd(s + 1) if s + 1 < NS else None
                    r0 = s * SR
                    for ocb in range(NB):
                        mps = [
                            psum.tile([128, SR, NT], F32, tag="ps", name=f"fm_{s}_{ocb}_{u}")
                            for u in range(4)
                        ]
                        k = 0
                        for dy in range(3):
                            for icb in range(NB):
                                for u in range(4):
                                    nc.tensor.matmul(
                                        mps[u],
                                        lhsT=fu_tile(u, dy, icb, ocb),
                                        rhs=vq[icb][:, u, dy : dy + SR, :],
                                        start=(k == 0),
                                        stop=(k == 5),
                                    )
                                k += 1
                        msb = mmp.tile([128, 4, SR, NT], BF16, tag="m", name=f"fmsb_{s}_{ocb}")
                        for u in range(4):
                            if u == 1:
                                nc.scalar.activation(
                                    out=msb[:, 1], in_=mps[1],
                                    func=IDENT, bias=convbf[ocb],
                                )
                            else:
                                nc.scalar.copy(out=msb[:, u], in_=mps[u])
                        t0 = mmp.tile([128, SR, NT], BF16, tag="t0", name=f"ft0_{s}_{ocb}")
                        nc.vector.tensor_add(out=t0, in0=msb[:, 0], in1=msb[:, 1])
                        t1 = mmp.tile([128, SR, NT], BF16, tag="t1", name=f"ft1_{s}_{ocb}")
                        nc.vector.tensor_sub(out=t1, in0=msb[:, 1], in1=msb[:, 2])
                        ost = ostp.tile([128, SR, 2, NT], BF16, tag="ost", name=f"ost_{s}_{ocb}")
                        nc.vector.tensor_add(
                            out=ost[:, :, 0, :], in0=t0, in1=msb[:, 2]
                        )
                        nc.vector.tensor_sub(
                            out=ost[:, :, 1, :], in0=t1, in1=msb[:, 3]
                        )
                        nc.gpsimd.dma_start(
                            out=out_d[ocb][:, r0 : r0 + SR, :, :], in_=ost
                        )
                    vq = vn

    _dedup_ldweights(nc)
    _split_waits(nc)
    return nc


def _dedup_ldweights(nc):
    """Drop InstLdweights that reload the exact weights already resident in
    the PE array. Self-loading matmuls (ldweights=None, fp32) invalidate the
    tracked state."""
    n_drop = 0
    for f in nc.m.functions:
        for bb in f.blocks:
            cur = None
            new_insts = []
            changed = False
            for inst in bb.instructions:
                t = type(inst).__name__
                if t == "InstLdweights":
                    si = inst.sync_info
                    clean = not (si and (si.on_wait or si.on_update))
                    key = str(inst.ins[0])
                    if clean and cur == key:
                        n_drop += 1
                        changed = True
                        continue
                    cur = key
                elif t == "InstMatmult" and inst.ldweights is not False:
                    cur = None  # self-loading matmul clobbers array weights
                new_insts.append(inst)
            if changed:
                bb.instructions = new_insts
    return n_drop


def _split_waits(nc, max_waits=1):
    """Walrus codegen allows only one embedded sync-wait per instruction
    (except SyncE drains). Move excess waits onto injected same-engine NOPs
    placed immediately before the over-constrained instruction."""
    n_new = 0
    for f in nc.m.functions:
        for bb in f.blocks:
            new_insts = []
            changed = False
            for inst in bb.instructions:
                si = inst.sync_info
                if si is not None and si.on_wait and len(si.on_wait) > max_waits:
                    extra = list(si.on_wait)[:-max_waits]
                    keep = list(si.on_wait)[-max_waits:]
                    for w in extra:
                        nop = mybir.InstNoOp(name=f"waitnop-{n_new}", ins=[], outs=[])
                        nop.engine = inst.engine
                        nop.sync_info = mybir.SyncInfo(on_wait=[w], on_update=[])
                        new_insts.append(nop)
                        n_new += 1
                    inst.sync_info = mybir.SyncInfo(
                        on_wait=keep, on_update=list(si.on_update)
                    )
                    changed = True
                new_insts.append(inst)
            if changed:
                bb.instructions = new_insts
    return n_new


def _prep_inputs(x, w_spatial, w_pointwise, bias, conv_w, conv_b):
    """Layout-only host prep: shard + transpose/scatter weights."""
    bf16 = ml_dtypes.bfloat16
    x = np.asarray(x, np.float32)
    w_spatial = np.asarray(w_spatial, np.float32)
    w_pointwise = np.asarray(w_pointwise, np.float32)
    bias = np.asarray(bias, np.float32)
    conv_w = np.asarray(conv_w, np.float32)
    conv_b = np.asarray(conv_b, np.float32)

    # cwt[icb, ic, off, ocb, oc] = conv_w[ocb*128+oc, icb*128+ic, off]
    cw = conv_w.reshape(C, C, NOFF)
    cwt = np.ascontiguousarray(
        cw.reshape(NB, 128, NB, 128, NOFF).transpose(2, 3, 4, 0, 1)
    ).astype(bf16)
    convbp = np.ascontiguousarray(conv_b.reshape(NB, 128, 1), np.float32)

    in_maps = []
    for b in range(B):
        ws = w_spatial[b].reshape(C, 8, NOFF)  # [o_glob, j_local, off]
        wsbd = np.zeros((NB, 128, NOFF, 128), np.float32)
        t = wsbd.reshape(NB, 16, 8, NOFF, 16, 8)
        wsv = ws.reshape(NB, 16, 8, 8, NOFF)  # [cb, g, oo, jj, off]
        for g in range(16):
            t[:, g, :, :, g, :] = wsv[:, g].transpose(0, 1, 3, 2)  # [cb, oo, off, jj]
        wp = w_pointwise[b][:, :, 0, 0].reshape(NB, 16, 8, 8)  # [cb, g, oo, ii]
        wptbd = np.zeros((NB, 128, 128), np.float32)
        t2 = wptbd.reshape(NB, 16, 8, 16, 8)
        for g in range(16):
            t2[:, g, :, g, :] = wp[:, g].transpose(0, 2, 1)  # [cb, ii, oo]
        wcat = np.concatenate(
            [
                wsbd.reshape(NB, 128, NOFF * 128),
                wptbd,
                np.ascontiguousarray(bias[b].reshape(NB, 128, 1)),
                convbp,
            ],
            axis=2,
        )
        in_maps.append(
            {
                "x": np.ascontiguousarray(x[b].reshape(C, HW)),
                "wcat": np.ascontiguousarray(wcat).astype(bf16),
                "cwt": cwt,
            }
        )
    return in_maps


def kernel(x, w_spatial, w_pointwise, bias, conv_w, conv_b):
    global LAST_EXEC_NS
    if "nc" not in _CACHE:
        _CACHE["nc"] = _build()
    nc = _CACHE["nc"]
    in_maps = _prep_inputs(x, w_spatial, w_pointwise, bias, conv_w, conv_b)
    res = run_bass_kernel_spmd(nc, in_maps, core_ids=list(range(B)))
    LAST_EXEC_NS = res.exec_time_ns
    # out_d [NB, 128, H, 2, NT] -> [B, C, H, W] (reinterleave x parity)
    outs = []
    for r in res.results:
        o = np.asarray(r["out"])  # bf16
        o = o.transpose(0, 1, 2, 4, 3).reshape(C, H, W)
        outs.append(o)
    return np.stack(outs).astype(np.float32)
